# revision 35
# baseline (speedup 1.0000x reference)
"""Trainium2 Bass kernel for a gated bilinear-attention GNN (GAT-with-gate).

Math (per batch b):
    h   = x @ W_w.T + W_b                      [N, D]
    e   = (h A) h^T ; e_sym = e + e^T = h (A + A^T) h^T   (one quadratic form)
    m   = where(adj > 0, e_sym, 0)
    att = softmax(m, axis=1) * adj             (masked entries contribute exp(0)=1
                                                to the denominator, then re-masked)
    rv  = h; 3x: az = relu(att @ rv);  c = sigmoid([h, az] @ gate_w.T + gate_b)
               rv = c * h + (1 - c) * az

Device strategy: data-parallel over the batch dim, 2 batches per core on 8
cores.  All large tensors live in a transposed layout [j(=contraction/softmax
column), i] so the softmax denominator is a per-partition (free-axis) reduction
and the hop matmul az^T = rv^T-style contraction streams at full float32r rate:

    attT[j, i] = adj[i, j] * exp(e_sym[j, i])   unnormalized, built as
                 exp((e+C)*adjT - C) so masked entries underflow to ~1e-26
    denom[j]   = exp-accum row sums + (N - indeg[j]) metadata
    azT[f, i]  = sum_j (rv[j, f]/denom[j]) * attT[j, i]   (normalization and
                 the gate coefficients folded into the stationary operand)

The sigmoid gate is evaluated as 1/(1+exp(-x)) to keep every ScalarE
activation in one LUT set (no ACT table reloads).  The two batches per core
are traced phase-interleaved so each batch's matmul bursts fill the other's
gate/combine gaps.  _fixup_waits post-processes the scheduled program to
satisfy this walrus build's one-sync-wait-per-instruction limit.

Host side only re-lays-out inputs (shard, transpose, degree metadata).
"""

import sys
from contextlib import ExitStack

import numpy as np

sys.path.insert(0, "/opt/trn_rl_repo")

import concourse.bass as bass
import concourse.tile as tile
from concourse import mybir
from concourse.bass_utils import run_bass_kernel_spmd


B, N, D = 16, 1024, 128
NCORES = 8
BPC = B // NCORES        # batches per core
NB = N // 128            # 128-row blocks per matrix dim
F32 = mybir.dt.float32
F32R = mybir.dt.float32r
OP = mybir.AluOpType
AF = mybir.ActivationFunctionType
CBIG = 60.0




def build_nc():
    nc = bass.Bass("TRN2", target_bir_lowering=False, debug=False,
                   num_devices=NCORES)

    adjT = nc.dram_tensor("adjT", [BPC, N, N], F32, kind="ExternalInput").ap()
    xT = nc.dram_tensor("xT", [BPC, D, N], F32, kind="ExternalInput").ap()
    ndegT = nc.dram_tensor("ndegT", [BPC, D, NB], F32, kind="ExternalInput").ap()
    WwT = nc.dram_tensor("WwT", [D, D], F32, kind="ExternalInput").ap()
    Wb = nc.dram_tensor("Wb", [D, 1], F32, kind="ExternalInput").ap()
    Amat = nc.dram_tensor("Amat", [D, D], F32, kind="ExternalInput").ap()
    gwc = nc.dram_tensor("gwc", [D, 2], F32, kind="ExternalInput").ap()
    gbv = nc.dram_tensor("gbv", [1, 1], F32, kind="ExternalInput").ap()
    identd = nc.dram_tensor("identd", [128, 128], F32, kind="ExternalInput").ap()
    out = nc.dram_tensor("out", [BPC, N, D], F32, kind="ExternalOutput").ap()

    with tile.TileContext(nc) as tc, ExitStack() as ctx:
        consts = ctx.enter_context(tc.tile_pool(name="consts", bufs=1))
        ps_a = ctx.enter_context(tc.tile_pool(name="ps_a", bufs=4, space="PSUM"))
        ps_az = ps_a
        ps_tr = ctx.enter_context(tc.tile_pool(name="ps_tr", bufs=2, space="PSUM"))
        ps_g = ctx.enter_context(tc.tile_pool(name="ps_g", bufs=1, space="PSUM"))
        ps_ct = ctx.enter_context(tc.tile_pool(name="ps_ct", bufs=1, space="PSUM"))
        adj_pool = ctx.enter_context(tc.tile_pool(name="adj", bufs=6))
        att_pool = ctx.enter_context(tc.tile_pool(name="att", bufs=2))
        work = ctx.enter_context(tc.tile_pool(name="work", bufs=2))
        hop = ctx.enter_context(tc.tile_pool(name="hop", bufs=4))

        ident = consts.tile([128, 128], F32, tag="ident")
        nc.sync.dma_start(ident[:, :], identd[:, :])
        wwT_sb = consts.tile([D, D], F32, tag="wwT")
        nc.sync.dma_start(wwT_sb[:, :], WwT[:, :])
        wb_sb = consts.tile([D, 1], F32, tag="wb")
        nc.sync.dma_start(wb_sb[:, :], Wb[:, :])
        a_sb = consts.tile([D, D], F32, tag="amat")
        nc.sync.dma_start(a_sb[:, :], Amat[:, :])
        gwc_sb = consts.tile([D, 2], F32, tag="gwc")
        nc.sync.dma_start(gwc_sb[:, :], gwc[:, :])
        gb_sb = consts.tile([1, 1], F32, tag="gb")
        nc.sync.dma_start(gb_sb[:, :], gbv[:, :])
        negc_sb = consts.tile([128, 1], F32, tag="negc")
        nc.vector.memset(negc_sb[:, :], -CBIG)
        ngb_sb = consts.tile([1, 1], F32, tag="ngb")
        nc.vector.tensor_scalar(ngb_sb[:, :], gb_sb[:, :], -1.0, None, OP.mult)

        identr = consts.tile([128, 128], F32R, tag="identr")
        nc.vector.tensor_copy(identr[:, :], ident[:, :])
        gwr_sb = consts.tile([D, 2], F32R, tag="gwr")
        nc.vector.tensor_copy(gwr_sb[:, :], gwc_sb[:, :])

        # PE warm-up: ~4us of tiny filler transposes during the otherwise
        # idle DMA-bound startup, so the HAM clock gate is already at 2.4GHz
        # when the first real matmuls issue.
        warm_ps = ps_ct.tile([128, NB], F32, tag="ps_ct")
        for _ in range(20):
            nc.tensor.transpose(warm_ps[:, 0:8], ident[0:8, :], ident[0:8, 0:8])

        # S = A + A^T (stays for the whole kernel)
        s_sb = consts.tile([D, D], F32R, tag="smat")
        at_ps = ps_tr.tile([128, 512], F32, tag="ps_tr")
        nc.tensor.transpose(at_ps[:, 0:128], a_sb[:, :], ident[:, :])
        nc.vector.tensor_tensor(s_sb[:, :], a_sb[:, :], at_ps[:, 0:128], OP.add)

        def phase_prologue(b, st):
            xT_sb = work.tile([D, N], F32, tag="xT")
            for ih in range(2):
                nc.sync.dma_start(xT_sb[:, ih * 512:(ih + 1) * 512],
                                  xT[b, :, ih * 512:(ih + 1) * 512])
            ndeg_sb = work.tile([D, NB], F32, tag="ndeg")
            nc.sync.dma_start(ndeg_sb[:, :], ndegT[b, :, :])

            # hT[o, n] = sum_d WwT[d, o] xT[d, n] + Wb[o]  (plain fp32
            # matmul: rhs comes straight from DMA, off the startup path)
            hT_sb = work.tile([D, N], F32R, tag="hT")
            for ih in range(2):
                ph = ps_a.tile([128, 512], F32, tag="ps_a")
                nc.tensor.matmul(ph[:, :], (wwT_sb[:, :]),
                                 (xT_sb[:, ih * 512:(ih + 1) * 512]),
                                 start=True, stop=True)
                nc.scalar.activation(hT_sb[:, ih * 512:(ih + 1) * 512], ph[:, :],
                                     AF.Identity, bias=wb_sb[:, :], scale=1.0)

            # hST[e, n] = sum_o S[o, e] hT[o, n]   (S symmetric)
            hST_sb = work.tile([D, N], F32R, tag="hST")
            for ih in range(2):
                ph = ps_a.tile([128, 512], F32, tag="ps_a")
                nc.tensor.matmul(ph[:, :], (s_sb[:, :]),
                                 (hT_sb[:, ih * 512:(ih + 1) * 512]),
                                 start=True, stop=True)
                nc.scalar.copy(hST_sb[:, ih * 512:(ih + 1) * 512], ph[:, :])

            # h in natural layout [node-in-block, nb*128 + f]
            hnat_sb = work.tile([128, N], F32, tag="hnat")
            for half in range(2):
                pt = ps_tr.tile([128, 512], F32R, tag="ps_tr")
                for q in range(4):
                    nb = half * 4 + q
                    nc.tensor.transpose(pt[:, q * 128:(q + 1) * 128],
                                        hT_sb[:, nb * 128:(nb + 1) * 128],
                                        identr[:, :])
                nc.scalar.copy(hnat_sb[:, half * 512:(half + 1) * 512],
                               pt[:, :])
            st.update(hT=hT_sb, hST=hST_sb, hnat=hnat_sb, ndeg=ndeg_sb)

        def phase_att(b, st):
            # attT = adj^T * exp(e_sym) via the masked-offset trick:
            # m = (e + C)*adjT, then exp(m - C).  Unmasked entries give
            # exp(e); masked give exp(-C) ~ 1e-26 ~ 0.  The exp's fused
            # accum_out yields sum_i over unmasked entries; reference
            # semantics add exp(0)=1 per masked entry, supplied as N-deg
            # metadata (ndegT).
            hT_sb, hST_sb = st["hT"], st["hST"]
            attT_sb = att_pool.tile([128, NB * N], F32R, tag="att")
            acc_sb = work.tile([D, NB], F32, tag="acc")
            for jb in range(NB):
                adj_sb = adj_pool.tile([128, N], F32, tag="adj")
                for ih in range(2):
                    nc.sync.dma_start(
                        adj_sb[:, ih * 512:(ih + 1) * 512],
                        adjT[b, jb * 128:(jb + 1) * 128,
                             ih * 512:(ih + 1) * 512])
                for ih in range(2):
                    pe = ps_a.tile([128, 512], F32, tag="ps_a")
                    nc.tensor.matmul(pe[:, :],
                                     (hST_sb[:, jb * 128:(jb + 1) * 128]),
                                     (hT_sb[:, ih * 512:(ih + 1) * 512]),
                                     start=True, stop=True)
                    seg = attT_sb[:, jb * N + ih * 512: jb * N + (ih + 1) * 512]
                    nc.vector.scalar_tensor_tensor(
                        seg, pe[:, :], CBIG,
                        adj_sb[:, ih * 512:(ih + 1) * 512],
                        OP.add, OP.mult)
                slab = attT_sb[:, jb * N:(jb + 1) * N]
                nc.scalar.activation(slab, slab, AF.Exp, bias=negc_sb[:, :],
                                     accum_out=acc_sb[:, jb:jb + 1])
                # filler transposes: hold the HAM clock warm through the
                # E phase's ~50% PE duty cycle
                for _ in range(4):
                    nc.tensor.transpose(warm_ps[:, 0:8], ident[0:8, :],
                                        ident[0:8, 0:8])

            # denom = masked-exp row sums + (N - deg);  inv = 1/denom
            inv_sb = work.tile([D, NB], F32, tag="inv")
            nc.vector.tensor_tensor(inv_sb[:, :], acc_sb[:, :],
                                    st["ndeg"][:, :], OP.add)
            nc.vector.reciprocal(inv_sb[:, :], inv_sb[:, :])

            # rv scaled by 1/denom for the first hop's stationary operand
            rvs = hop.tile([128, N], F32R, tag="rvs")
            hnat_sb = st["hnat"]
            for nb in range(NB):
                nc.vector.tensor_scalar_mul(rvs[:, nb * 128:(nb + 1) * 128],
                                            hnat_sb[:, nb * 128:(nb + 1) * 128],
                                            inv_sb[:, nb:nb + 1])
            st.update(att=attT_sb, inv=inv_sb, rvs=rvs)

        def phase_hop(b, st, k):
            last = (k == 2)
            hT_sb, hnat_sb = st["hT"], st["hnat"]
            attT_sb, inv_sb, rvs = st["att"], st["inv"], st["rvs"]
            # azT[f, i] = sum_j rvs[j, f] attT[j, i]
            azT_sb = hop.tile([128, N], F32R, tag="azT")
            for ih in range(2):
                paz = ps_az.tile([128, 512], F32, tag="ps_a")
                for jb in range(NB):
                    nc.tensor.matmul(
                        paz[:, :], (rvs[:, jb * 128:(jb + 1) * 128]),
                        (attT_sb[:, jb * N + ih * 512: jb * N + (ih + 1) * 512]),
                        start=(jb == 0), stop=(jb == NB - 1))
                nc.scalar.activation(azT_sb[:, ih * 512:(ih + 1) * 512],
                                     paz[:, :], AF.Relu)

            # gate: coeff = sigmoid(gw1.h + gw2.az + gb) per node, computed
            # as 1/(1 + exp(-pre)) to stay in the exp LUT set (a Sigmoid
            # activation would force an ACT table swap).
            en_sb = hop.tile([1, N], F32, tag="coeff")
            for ih in range(2):
                pg = ps_g.tile([1, 512], F32, tag="ps_g")
                nc.tensor.matmul(pg[:, :], (gwr_sb[:, 0:1]),
                                 (hT_sb[:, ih * 512:(ih + 1) * 512]),
                                 start=True, stop=False)
                nc.tensor.matmul(pg[:, :], (gwr_sb[:, 1:2]),
                                 (azT_sb[:, ih * 512:(ih + 1) * 512]),
                                 start=False, stop=True)
                nc.scalar.activation(en_sb[:, ih * 512:(ih + 1) * 512],
                                     pg[:, :], AF.Exp, bias=ngb_sb[:, :],
                                     scale=-1.0)

            # transpose exp(-pre) to per-partition scalars, finish the
            # sigmoid there (tiny [128, NB] ops)
            ct_ps = ps_ct.tile([128, NB], F32, tag="ps_ct")
            for nb in range(NB):
                nc.tensor.transpose(ct_ps[:, nb:nb + 1],
                                    en_sb[0:1, nb * 128:(nb + 1) * 128],
                                    ident[0:1, 0:1])
            # coeff c = 1/(1+e); w1 = c (*1/denom unless last),
            # w2 = 1-c = e*c (*1/denom unless last)
            w1 = hop.tile([128, NB], F32, tag="w1")
            w2 = hop.tile([128, NB], F32, tag="w2")
            nc.vector.tensor_scalar(w1[:, :], ct_ps[:, :], 1.0, None, OP.add)
            nc.vector.reciprocal(w1[:, :], w1[:, :])
            nc.vector.tensor_tensor(w2[:, :], ct_ps[:, :], w1[:, :], OP.mult)
            if not last:
                nc.vector.tensor_tensor(w1[:, :], w1[:, :], inv_sb[:, :],
                                        OP.mult)
                nc.vector.tensor_tensor(w2[:, :], w2[:, :], inv_sb[:, :],
                                        OP.mult)

            # az back to natural layout, scale by w2, combine with h
            rv_new = hop.tile([128, N], F32 if last else F32R, tag="rvs")
            azs = hop.tile([128, N], F32, tag="azs")
            for half in range(2):
                pt = ps_tr.tile([128, 512], F32R, tag="ps_tr")
                for q in range(4):
                    nb = half * 4 + q
                    nc.tensor.transpose(pt[:, q * 128:(q + 1) * 128],
                                        azT_sb[:, nb * 128:(nb + 1) * 128],
                                        identr[:, :])
                for q in range(4):
                    nb = half * 4 + q
                    sl = slice(nb * 128, (nb + 1) * 128)
                    nc.vector.tensor_scalar_mul(
                        azs[:, sl], pt[:, q * 128:(q + 1) * 128],
                        w2[:, nb:nb + 1])
                    nc.vector.scalar_tensor_tensor(rv_new[:, sl],
                                                   hnat_sb[:, sl],
                                                   w1[:, nb:nb + 1],
                                                   azs[:, sl],
                                                   OP.mult, OP.add)
            if last:
                for nb in range(NB):
                    nc.sync.dma_start(out[b, nb * 128:(nb + 1) * 128, :],
                                      rv_new[:, nb * 128:(nb + 1) * 128])
            else:
                st["rvs"] = rv_new

        # Interleave the two batches phase-by-phase so each batch's PE-heavy
        # bursts fill the other batch's gate/combine gaps (keeps the PE HAM
        # clock warm and every engine fed).
        states = [{} for _ in range(BPC)]
        for b in range(BPC):
            phase_prologue(b, states[b])
        for b in range(BPC):
            phase_att(b, states[b])
        for k in range(3):
            for b in range(BPC):
                phase_hop(b, states[b], k)

        # Spare per-engine nops: relocated by _fixup_waits to carry sync
        # waits that walrus cannot fit on compute-instruction structs.
        nop_insts = []
        for eng in (nc.tensor, nc.vector, nc.scalar, nc.gpsimd, nc.sync):
            for _ in range(96):
                nop_insts.append(eng.nop(nofuse=True).ins)

    _fixup_waits(nc, nop_insts)
    return nc


_FIXUP_SKIP = {"InstNoOp"}


def _fixup_waits(nc, nop_insts):
    """walrus (enable-ldw-opt=false) rejects compute instructions with more
    than one sync wait (single wait slot in the S3 structs).  Hoist
    all-but-one wait of each such instruction onto spare same-engine nop
    instructions inserted immediately before it in program order."""
    nop_set = set(id(x) for x in nop_insts)
    free_nops = {}
    for x in nop_insts:
        free_nops.setdefault(x.engine, []).append(x)
    f = nc.m.functions[0]
    for blk in f.blocks:
        insts = blk.instructions
        for i in range(len(insts) - 1, -1, -1):
            if id(insts[i]) in nop_set:
                insts.pop(i)
        i = 0
        while i < len(insts):
            inst = insts[i]
            if inst.__class__.__name__ not in _FIXUP_SKIP:
                si = inst.sync_info
                if si is not None and si.on_wait and len(si.on_wait) > 1:
                    waits = list(si.on_wait)
                    extra, keep = waits[:-1], waits[-1:]
                    inst.sync_info = mybir.SyncInfo(
                        on_wait=keep, on_update=list(si.on_update or []))
                    pool = free_nops.get(inst.engine)
                    for k, w in enumerate(extra):
                        if not pool:
                            raise RuntimeError(
                                f"out of spare nops for {inst.engine}")
                        nop = pool.pop()
                        nop.sync_info = mybir.SyncInfo(on_wait=[w], on_update=[])
                        insts.insert(i + k, nop)
                    i += len(extra)
            i += 1


_NC_CACHE = None


def _get_nc():
    global _NC_CACHE
    if _NC_CACHE is None:
        _NC_CACHE = build_nc()
    return _NC_CACHE


def _prep_in_maps(inputs):
    x = np.ascontiguousarray(np.asarray(inputs["x"], dtype=np.float32))
    adj = np.ascontiguousarray(np.asarray(inputs["adj"], dtype=np.float32))
    W_w = np.asarray(inputs["W_w"], dtype=np.float32)
    W_b = np.asarray(inputs["W_b"], dtype=np.float32)
    A = np.asarray(inputs["A"], dtype=np.float32)
    gate_w = np.asarray(inputs["gate_w"], dtype=np.float32)
    gate_b = np.asarray(inputs["gate_b"], dtype=np.float32)

    WwT = np.ascontiguousarray(W_w.T)
    Wb2 = np.ascontiguousarray(W_b.reshape(D, 1))
    gwcols = np.ascontiguousarray(gate_w.reshape(2, D).T)
    gb2 = np.ascontiguousarray(gate_b.reshape(1, 1))
    ident128 = np.eye(128, dtype=np.float32)

    in_maps = []
    for c in range(NCORES):
        sl = slice(c * BPC, (c + 1) * BPC)
        adj_c = adj[sl]
        adjT_c = np.ascontiguousarray(adj_c.transpose(0, 2, 1))
        xT_c = np.ascontiguousarray(x[sl].transpose(0, 2, 1))
        ndeg = (N - adj_c.sum(axis=1)).astype(np.float32)          # [BPC, N]
        ndegT = np.ascontiguousarray(
            ndeg.reshape(BPC, NB, 128).transpose(0, 2, 1))         # [BPC, 128, NB]
        in_maps.append({
            "adjT": adjT_c, "xT": xT_c, "ndegT": ndegT,
            "WwT": WwT, "Wb": Wb2, "Amat": np.ascontiguousarray(A),
            "gwc": gwcols, "gbv": gb2, "identd": ident128,
        })
    return in_maps


def _run(inputs, trace=False, **kwargs):
    nc = _get_nc()
    in_maps = _prep_in_maps(inputs)
    res = run_bass_kernel_spmd(nc, in_maps, core_ids=list(range(NCORES)),
                               trace=trace, **kwargs)
    out = np.concatenate([res.results[c]["out"] for c in range(NCORES)], axis=0)
    return out.astype(np.float32), res


def kernel(**inputs) -> np.ndarray:
    out, _ = _run(inputs, trace=False)
    return out


# revision 36
# speedup vs baseline: 1.0513x; 1.0513x over previous
"""Trainium2 Bass kernel for a gated bilinear-attention GNN (GAT-with-gate).

Math (per batch b):
    h   = x @ W_w.T + W_b                      [N, D]
    e   = (h A) h^T ; e_sym = e + e^T = h (A + A^T) h^T   (one quadratic form)
    m   = where(adj > 0, e_sym, 0)
    att = softmax(m, axis=1) * adj             (masked entries contribute exp(0)=1
                                                to the denominator, then re-masked)
    rv  = h; 3x: az = relu(att @ rv);  c = sigmoid([h, az] @ gate_w.T + gate_b)
               rv = c * h + (1 - c) * az

Device strategy: data-parallel over the batch dim, 2 batches per core on 8
cores.  All large tensors live in a transposed layout [j(=contraction/softmax
column), i] so the softmax denominator is a per-partition (free-axis) reduction
and the hop matmul az^T = rv^T-style contraction streams at full float32r rate:

    attT[j, i] = adj[i, j] * exp(e_sym[j, i])   unnormalized, built as
                 exp((e+C)*adjT - C) so masked entries underflow to ~1e-26
    denom[j]   = exp-accum row sums + (N - indeg[j]) metadata
    azT[f, i]  = sum_j (rv[j, f]/denom[j]) * attT[j, i]   (normalization and
                 the gate coefficients folded into the stationary operand)

The sigmoid gate is evaluated as 1/(1+exp(-x)) to keep every ScalarE
activation in one LUT set (no ACT table reloads).  The two batches per core
are traced phase-interleaved so each batch's matmul bursts fill the other's
gate/combine gaps.  _fixup_waits post-processes the scheduled program to
satisfy this walrus build's one-sync-wait-per-instruction limit.

Host side only re-lays-out inputs (shard, transpose, degree metadata).
"""

import sys
from contextlib import ExitStack

import numpy as np

sys.path.insert(0, "/opt/trn_rl_repo")

import concourse.bass as bass
import concourse.tile as tile
from concourse import mybir
from concourse.bass_utils import run_bass_kernel_spmd


B, N, D = 16, 1024, 128
NCORES = 8
BPC = B // NCORES        # batches per core
NB = N // 128            # 128-row blocks per matrix dim
F32 = mybir.dt.float32
F32R = mybir.dt.float32r
OP = mybir.AluOpType
AF = mybir.ActivationFunctionType
CBIG = 60.0




def build_nc():
    nc = bass.Bass("TRN2", target_bir_lowering=False, debug=False,
                   num_devices=NCORES)

    adjT = nc.dram_tensor("adjT", [BPC, N, N], F32, kind="ExternalInput").ap()
    xT = nc.dram_tensor("xT", [BPC, D, N], F32, kind="ExternalInput").ap()
    ndegT = nc.dram_tensor("ndegT", [BPC, D, NB], F32, kind="ExternalInput").ap()
    WwT = nc.dram_tensor("WwT", [D, D], F32, kind="ExternalInput").ap()
    Wb = nc.dram_tensor("Wb", [D, 1], F32, kind="ExternalInput").ap()
    Amat = nc.dram_tensor("Amat", [D, D], F32, kind="ExternalInput").ap()
    gwc = nc.dram_tensor("gwc", [D, 2], F32, kind="ExternalInput").ap()
    gbv = nc.dram_tensor("gbv", [1, 1], F32, kind="ExternalInput").ap()
    identd = nc.dram_tensor("identd", [128, 128], F32, kind="ExternalInput").ap()
    out = nc.dram_tensor("out", [BPC, N, D], F32, kind="ExternalOutput").ap()

    with tile.TileContext(nc) as tc, ExitStack() as ctx:
        consts = ctx.enter_context(tc.tile_pool(name="consts", bufs=1))
        ps_a = ctx.enter_context(tc.tile_pool(name="ps_a", bufs=4, space="PSUM"))
        ps_az = ps_a
        ps_tr = ctx.enter_context(tc.tile_pool(name="ps_tr", bufs=2, space="PSUM"))
        ps_g = ctx.enter_context(tc.tile_pool(name="ps_g", bufs=1, space="PSUM"))
        ps_ct = ctx.enter_context(tc.tile_pool(name="ps_ct", bufs=1, space="PSUM"))
        adj_pool = ctx.enter_context(tc.tile_pool(name="adj", bufs=6))
        att_pool = ctx.enter_context(tc.tile_pool(name="att", bufs=2))
        work = ctx.enter_context(tc.tile_pool(name="work", bufs=2))
        hop = ctx.enter_context(tc.tile_pool(name="hop", bufs=4))

        ident = consts.tile([128, 128], F32, tag="ident")
        nc.sync.dma_start(ident[:, :], identd[:, :])
        wwT_sb = consts.tile([D, D], F32, tag="wwT")
        nc.sync.dma_start(wwT_sb[:, :], WwT[:, :])
        wb_sb = consts.tile([D, 1], F32, tag="wb")
        nc.sync.dma_start(wb_sb[:, :], Wb[:, :])
        a_sb = consts.tile([D, D], F32, tag="amat")
        nc.sync.dma_start(a_sb[:, :], Amat[:, :])
        gwc_sb = consts.tile([D, 2], F32, tag="gwc")
        nc.sync.dma_start(gwc_sb[:, :], gwc[:, :])
        gb_sb = consts.tile([1, 1], F32, tag="gb")
        nc.sync.dma_start(gb_sb[:, :], gbv[:, :])
        negc_sb = consts.tile([128, 1], F32, tag="negc")
        nc.vector.memset(negc_sb[:, :], -CBIG)
        ngb_sb = consts.tile([1, 1], F32, tag="ngb")
        nc.vector.tensor_scalar(ngb_sb[:, :], gb_sb[:, :], -1.0, None, OP.mult)

        identr = consts.tile([128, 128], F32R, tag="identr")
        nc.vector.tensor_copy(identr[:, :], ident[:, :])
        gwr_sb = consts.tile([D, 2], F32R, tag="gwr")
        nc.vector.tensor_copy(gwr_sb[:, :], gwc_sb[:, :])

        # PE warm-up: ~4us of tiny filler transposes during the otherwise
        # idle DMA-bound startup, so the HAM clock gate is already at 2.4GHz
        # when the first real matmuls issue.
        warm_ps = ps_ct.tile([128, NB], F32, tag="ps_ct")
        for _ in range(20):
            nc.tensor.transpose(warm_ps[:, 0:8], ident[0:8, :], ident[0:8, 0:8])

        # S = A + A^T (stays for the whole kernel)
        s_sb = consts.tile([D, D], F32R, tag="smat")
        at_ps = ps_tr.tile([128, 512], F32, tag="ps_tr")
        nc.tensor.transpose(at_ps[:, 0:128], a_sb[:, :], ident[:, :])
        nc.vector.tensor_tensor(s_sb[:, :], a_sb[:, :], at_ps[:, 0:128], OP.add)

        def phase_prologue(b, st):
            xT_sb = work.tile([D, N], F32, tag="xT")
            for ih in range(2):
                nc.sync.dma_start(xT_sb[:, ih * 512:(ih + 1) * 512],
                                  xT[b, :, ih * 512:(ih + 1) * 512])
            ndeg_sb = work.tile([D, NB], F32, tag="ndeg")
            nc.sync.dma_start(ndeg_sb[:, :], ndegT[b, :, :])

            # hT[o, n] = sum_d WwT[d, o] xT[d, n] + Wb[o]  (plain fp32
            # matmul: rhs comes straight from DMA, off the startup path)
            hT_sb = work.tile([D, N], F32R, tag="hT")
            for ih in range(2):
                ph = ps_a.tile([128, 512], F32, tag="ps_a")
                nc.tensor.matmul(ph[:, :], (wwT_sb[:, :]),
                                 (xT_sb[:, ih * 512:(ih + 1) * 512]),
                                 start=True, stop=True)
                nc.scalar.activation(hT_sb[:, ih * 512:(ih + 1) * 512], ph[:, :],
                                     AF.Identity, bias=wb_sb[:, :], scale=1.0)

            # hST[e, n] = sum_o S[o, e] hT[o, n]   (S symmetric)
            hST_sb = work.tile([D, N], F32R, tag="hST")
            for ih in range(2):
                ph = ps_a.tile([128, 512], F32, tag="ps_a")
                nc.tensor.matmul(ph[:, :], (s_sb[:, :]),
                                 (hT_sb[:, ih * 512:(ih + 1) * 512]),
                                 start=True, stop=True)
                nc.scalar.copy(hST_sb[:, ih * 512:(ih + 1) * 512], ph[:, :])

            # h in natural layout [node-in-block, nb*128 + f]
            hnat_sb = work.tile([128, N], F32, tag="hnat")
            for half in range(2):
                pt = ps_tr.tile([128, 512], F32R, tag="ps_tr")
                for q in range(4):
                    nb = half * 4 + q
                    nc.tensor.transpose(pt[:, q * 128:(q + 1) * 128],
                                        hT_sb[:, nb * 128:(nb + 1) * 128],
                                        identr[:, :])
                nc.scalar.copy(hnat_sb[:, half * 512:(half + 1) * 512],
                               pt[:, :])
            st.update(hT=hT_sb, hST=hST_sb, hnat=hnat_sb, ndeg=ndeg_sb)

        def phase_att(b, st):
            # attT = adj^T * exp(e_sym) via the masked-offset trick:
            # m = (e + C)*adjT, then exp(m - C).  Unmasked entries give
            # exp(e); masked give exp(-C) ~ 1e-26 ~ 0.  The exp's fused
            # accum_out yields sum_i over unmasked entries; reference
            # semantics add exp(0)=1 per masked entry, supplied as N-deg
            # metadata (ndegT).
            hT_sb, hST_sb = st["hT"], st["hST"]
            attT_sb = att_pool.tile([128, NB * N], F32R, tag="att")
            acc_sb = work.tile([D, NB], F32, tag="acc")
            for jb in range(NB):
                adj_sb = adj_pool.tile([128, N], F32, tag="adj")
                for ih in range(2):
                    nc.sync.dma_start(
                        adj_sb[:, ih * 512:(ih + 1) * 512],
                        adjT[b, jb * 128:(jb + 1) * 128,
                             ih * 512:(ih + 1) * 512])
                for ih in range(2):
                    pe = ps_a.tile([128, 512], F32, tag="ps_a")
                    nc.tensor.matmul(pe[:, :],
                                     (hST_sb[:, jb * 128:(jb + 1) * 128]),
                                     (hT_sb[:, ih * 512:(ih + 1) * 512]),
                                     start=True, stop=True)
                    seg = attT_sb[:, jb * N + ih * 512: jb * N + (ih + 1) * 512]
                    nc.vector.scalar_tensor_tensor(
                        seg, pe[:, :], CBIG,
                        adj_sb[:, ih * 512:(ih + 1) * 512],
                        OP.add, OP.mult)
                slab = attT_sb[:, jb * N:(jb + 1) * N]
                nc.scalar.activation(slab, slab, AF.Exp, bias=negc_sb[:, :],
                                     accum_out=acc_sb[:, jb:jb + 1])

            # denom = masked-exp row sums + (N - deg);  inv = 1/denom
            inv_sb = work.tile([D, NB], F32, tag="inv")
            nc.vector.tensor_tensor(inv_sb[:, :], acc_sb[:, :],
                                    st["ndeg"][:, :], OP.add)
            nc.vector.reciprocal(inv_sb[:, :], inv_sb[:, :])

            # rv scaled by 1/denom for the first hop's stationary operand
            rvs = hop.tile([128, N], F32R, tag="rvs")
            hnat_sb = st["hnat"]
            for nb in range(NB):
                nc.vector.tensor_scalar_mul(rvs[:, nb * 128:(nb + 1) * 128],
                                            hnat_sb[:, nb * 128:(nb + 1) * 128],
                                            inv_sb[:, nb:nb + 1])
            st.update(att=attT_sb, inv=inv_sb, rvs=rvs)

        def phase_hop(b, st, k):
            last = (k == 2)
            hT_sb, hnat_sb = st["hT"], st["hnat"]
            attT_sb, inv_sb, rvs = st["att"], st["inv"], st["rvs"]
            # azT[f, i] = sum_j rvs[j, f] attT[j, i]
            azT_sb = hop.tile([128, N], F32R, tag="azT")
            for ih in range(2):
                paz = ps_az.tile([128, 512], F32, tag="ps_a")
                for jb in range(NB):
                    nc.tensor.matmul(
                        paz[:, :], (rvs[:, jb * 128:(jb + 1) * 128]),
                        (attT_sb[:, jb * N + ih * 512: jb * N + (ih + 1) * 512]),
                        start=(jb == 0), stop=(jb == NB - 1))
                nc.scalar.activation(azT_sb[:, ih * 512:(ih + 1) * 512],
                                     paz[:, :], AF.Relu)

            # gate: coeff = sigmoid(gw1.h + gw2.az + gb) per node, computed
            # as 1/(1 + exp(-pre)) to stay in the exp LUT set (a Sigmoid
            # activation would force an ACT table swap).
            en_sb = hop.tile([1, N], F32, tag="coeff")
            for ih in range(2):
                pg = ps_g.tile([1, 512], F32, tag="ps_g")
                nc.tensor.matmul(pg[:, :], (gwr_sb[:, 0:1]),
                                 (hT_sb[:, ih * 512:(ih + 1) * 512]),
                                 start=True, stop=False)
                nc.tensor.matmul(pg[:, :], (gwr_sb[:, 1:2]),
                                 (azT_sb[:, ih * 512:(ih + 1) * 512]),
                                 start=False, stop=True)
                nc.scalar.activation(en_sb[:, ih * 512:(ih + 1) * 512],
                                     pg[:, :], AF.Exp, bias=ngb_sb[:, :],
                                     scale=-1.0)

            # transpose exp(-pre) to per-partition scalars, finish the
            # sigmoid there (tiny [128, NB] ops)
            ct_ps = ps_ct.tile([128, NB], F32, tag="ps_ct")
            for nb in range(NB):
                nc.tensor.transpose(ct_ps[:, nb:nb + 1],
                                    en_sb[0:1, nb * 128:(nb + 1) * 128],
                                    ident[0:1, 0:1])
            # coeff c = 1/(1+e); w1 = c (*1/denom unless last),
            # w2 = 1-c = e*c (*1/denom unless last)
            w1 = hop.tile([128, NB], F32, tag="w1")
            w2 = hop.tile([128, NB], F32, tag="w2")
            nc.vector.tensor_scalar(w1[:, :], ct_ps[:, :], 1.0, None, OP.add)
            nc.vector.reciprocal(w1[:, :], w1[:, :])
            nc.vector.tensor_tensor(w2[:, :], ct_ps[:, :], w1[:, :], OP.mult)
            if not last:
                nc.vector.tensor_tensor(w1[:, :], w1[:, :], inv_sb[:, :],
                                        OP.mult)
                nc.vector.tensor_tensor(w2[:, :], w2[:, :], inv_sb[:, :],
                                        OP.mult)

            # az back to natural layout, scale by w2, combine with h
            rv_new = hop.tile([128, N], F32 if last else F32R, tag="rvs")
            azs = hop.tile([128, N], F32, tag="azs")
            for half in range(2):
                pt = ps_tr.tile([128, 512], F32R, tag="ps_tr")
                for q in range(4):
                    nb = half * 4 + q
                    nc.tensor.transpose(pt[:, q * 128:(q + 1) * 128],
                                        azT_sb[:, nb * 128:(nb + 1) * 128],
                                        identr[:, :])
                for q in range(4):
                    nb = half * 4 + q
                    sl = slice(nb * 128, (nb + 1) * 128)
                    nc.vector.tensor_scalar_mul(
                        azs[:, sl], pt[:, q * 128:(q + 1) * 128],
                        w2[:, nb:nb + 1])
                    nc.vector.scalar_tensor_tensor(rv_new[:, sl],
                                                   hnat_sb[:, sl],
                                                   w1[:, nb:nb + 1],
                                                   azs[:, sl],
                                                   OP.mult, OP.add)
            if last:
                for nb in range(NB):
                    nc.sync.dma_start(out[b, nb * 128:(nb + 1) * 128, :],
                                      rv_new[:, nb * 128:(nb + 1) * 128])
            else:
                st["rvs"] = rv_new

        # Interleave the two batches phase-by-phase so each batch's PE-heavy
        # bursts fill the other batch's gate/combine gaps (keeps the PE HAM
        # clock warm and every engine fed).
        states = [{} for _ in range(BPC)]
        for b in range(BPC):
            phase_prologue(b, states[b])
        for b in range(BPC):
            phase_att(b, states[b])
        for k in range(3):
            for b in range(BPC):
                phase_hop(b, states[b], k)

        # Spare per-engine nops: relocated by _fixup_waits to carry sync
        # waits that walrus cannot fit on compute-instruction structs.
        nop_insts = []
        for eng in (nc.tensor, nc.vector, nc.scalar, nc.gpsimd, nc.sync):
            for _ in range(96):
                nop_insts.append(eng.nop(nofuse=True).ins)

    _fixup_waits(nc, nop_insts)
    return nc


_FIXUP_SKIP = {"InstNoOp"}


def _fixup_waits(nc, nop_insts):
    """walrus (enable-ldw-opt=false) rejects compute instructions with more
    than one sync wait (single wait slot in the S3 structs).  Hoist
    all-but-one wait of each such instruction onto spare same-engine nop
    instructions inserted immediately before it in program order."""
    nop_set = set(id(x) for x in nop_insts)
    free_nops = {}
    for x in nop_insts:
        free_nops.setdefault(x.engine, []).append(x)
    f = nc.m.functions[0]
    for blk in f.blocks:
        insts = blk.instructions
        for i in range(len(insts) - 1, -1, -1):
            if id(insts[i]) in nop_set:
                insts.pop(i)
        i = 0
        while i < len(insts):
            inst = insts[i]
            if inst.__class__.__name__ not in _FIXUP_SKIP:
                si = inst.sync_info
                if si is not None and si.on_wait and len(si.on_wait) > 1:
                    waits = list(si.on_wait)
                    extra, keep = waits[:-1], waits[-1:]
                    inst.sync_info = mybir.SyncInfo(
                        on_wait=keep, on_update=list(si.on_update or []))
                    pool = free_nops.get(inst.engine)
                    for k, w in enumerate(extra):
                        if not pool:
                            raise RuntimeError(
                                f"out of spare nops for {inst.engine}")
                        nop = pool.pop()
                        nop.sync_info = mybir.SyncInfo(on_wait=[w], on_update=[])
                        insts.insert(i + k, nop)
                    i += len(extra)
            i += 1


_NC_CACHE = None


def _get_nc():
    global _NC_CACHE
    if _NC_CACHE is None:
        _NC_CACHE = build_nc()
    return _NC_CACHE


def _prep_in_maps(inputs):
    x = np.ascontiguousarray(np.asarray(inputs["x"], dtype=np.float32))
    adj = np.ascontiguousarray(np.asarray(inputs["adj"], dtype=np.float32))
    W_w = np.asarray(inputs["W_w"], dtype=np.float32)
    W_b = np.asarray(inputs["W_b"], dtype=np.float32)
    A = np.asarray(inputs["A"], dtype=np.float32)
    gate_w = np.asarray(inputs["gate_w"], dtype=np.float32)
    gate_b = np.asarray(inputs["gate_b"], dtype=np.float32)

    WwT = np.ascontiguousarray(W_w.T)
    Wb2 = np.ascontiguousarray(W_b.reshape(D, 1))
    gwcols = np.ascontiguousarray(gate_w.reshape(2, D).T)
    gb2 = np.ascontiguousarray(gate_b.reshape(1, 1))
    ident128 = np.eye(128, dtype=np.float32)

    in_maps = []
    for c in range(NCORES):
        sl = slice(c * BPC, (c + 1) * BPC)
        adj_c = adj[sl]
        adjT_c = np.ascontiguousarray(adj_c.transpose(0, 2, 1))
        xT_c = np.ascontiguousarray(x[sl].transpose(0, 2, 1))
        ndeg = (N - adj_c.sum(axis=1)).astype(np.float32)          # [BPC, N]
        ndegT = np.ascontiguousarray(
            ndeg.reshape(BPC, NB, 128).transpose(0, 2, 1))         # [BPC, 128, NB]
        in_maps.append({
            "adjT": adjT_c, "xT": xT_c, "ndegT": ndegT,
            "WwT": WwT, "Wb": Wb2, "Amat": np.ascontiguousarray(A),
            "gwc": gwcols, "gbv": gb2, "identd": ident128,
        })
    return in_maps


def _run(inputs, trace=False, **kwargs):
    nc = _get_nc()
    in_maps = _prep_in_maps(inputs)
    res = run_bass_kernel_spmd(nc, in_maps, core_ids=list(range(NCORES)),
                               trace=trace, **kwargs)
    out = np.concatenate([res.results[c]["out"] for c in range(NCORES)], axis=0)
    return out.astype(np.float32), res


def kernel(**inputs) -> np.ndarray:
    out, _ = _run(inputs, trace=False)
    return out
